# revision 9
# baseline (speedup 1.0000x reference)
"""Local+global sparse attention (T=4096, D=64, window=512, global stride 64)
for Trainium2, sharded one head per NeuronCore (B*H = 8 = n_cores).

Per-head layout (all hardcoded for T=4096, D=64):
  - 8 query superblocks of 512 queries each.
  - Per superblock s: 8 band k-tiles of 128 keys covering k in
    [512(s-1), 512(s+1)) (only 4 tiles for s=0), masked via additive
    -1e30 masks applied to scores in PSUM, plus a "global" tile of the
    stride-64 keys k < 512(s-1) (all valid, no mask).
  - S^T layout [k_tile=128 part, q=512 free]: S = K_T.T @ Q_T (f32r).
  - exp on ScalarE (PSUM -> SBUF f32r), no max subtraction (scores are
    O(5) for randn inputs, exp stays finite in fp32).
  - PV: out^T[65, 512] += V_ext[k,65].T-style matmul accumulation where
    V_ext has a ones column producing the softmax denominator Z in row 64.
  - Host divides by Z and transposes back.
"""

import sys

sys.path.insert(0, "/opt/trn_rl_repo")

from contextlib import ExitStack

import numpy as np

import concourse.bass as bass
import concourse.mybir as mybir
import concourse.tile as tile
from concourse import bacc
from concourse.bass_utils import run_bass_kernel_spmd

f32 = mybir.dt.float32
f32r = mybir.dt.float32r
AF = mybir.ActivationFunctionType

T, D = 4096, 64
W, GS = 512, 64
NSB = T // 512            # 8 superblocks
SCALE = 1.0 / 8.0         # 1/sqrt(D)
NEG = -1.0e30

# additive mask pack layout (columns of the [128, 1408] mask tensor):
#   M_low_j0 for j0=0..3 at offsets below, widths 512-128*j0
#   M_diag at offset 1280, width 128
_M_OFF = [0, 512, 896, 1152]
_M_DIAG = 1280
_M_COLS = 1408

TRACE = False
LAST_RESULT = None


def _build_masks():
    kk = np.arange(128)[:, None]
    m = np.zeros((128, _M_COLS), np.float32)
    for j0 in range(4):
        w = 512 - 128 * j0
        r = np.arange(128 * j0, 512)[None, :]
        valid = (r <= kk + 128 * j0) | (kk % GS == 0)
        m[:, _M_OFF[j0] : _M_OFF[j0] + w] = np.where(valid, 0.0, NEG)
    rr = np.arange(128)[None, :]
    m[:, _M_DIAG : _M_DIAG + 128] = np.where(rr >= kk, 0.0, NEG)
    return m


def _build_nc():
    nc = bacc.Bacc("TRN2", target_bir_lowering=False, debug=False, num_devices=8)
    qt_d = nc.dram_tensor("qt", [64, 4096], f32, kind="ExternalInput")
    kt_d = nc.dram_tensor("kt", [64, 4096], f32, kind="ExternalInput")
    ve_d = nc.dram_tensor("ve", [128, 32 * 65], f32, kind="ExternalInput")
    kg_d = nc.dram_tensor("kg", [64, 64], f32, kind="ExternalInput")
    vg_d = nc.dram_tensor("vg", [128, 65], f32, kind="ExternalInput")
    m_d = nc.dram_tensor("m", [128, _M_COLS], f32, kind="ExternalInput")
    o_d = nc.dram_tensor("o", [NSB, 65, 512], f32, kind="ExternalOutput")

    with tile.TileContext(nc) as tc:
        with ExitStack() as ctx:
            const = ctx.enter_context(tc.tile_pool(name="const", bufs=1))
            ep = ctx.enter_context(tc.tile_pool(name="ep", bufs=6))
            op = ctx.enter_context(tc.tile_pool(name="op", bufs=3))
            ps_s = ctx.enter_context(tc.tile_pool(name="ps_s", bufs=5, space="PSUM"))
            ps_g = ctx.enter_context(tc.tile_pool(name="ps_g", bufs=1, space="PSUM"))
            ps_o = ctx.enter_context(tc.tile_pool(name="ps_o", bufs=2, space="PSUM"))

            # --- load + round inputs to f32r (DVE is the only producer the
            # matmuls ever wait on for SBUF operands) ---
            m_t = const.tile([128, _M_COLS], f32, tag="m_t")
            nc.sync.dma_start(out=m_t[:], in_=m_d[:])

            rtiles = {}
            for name, dh, shape in [
                ("kt", kt_d, [64, 4096]),
                ("qt", qt_d, [64, 4096]),
                ("ve", ve_d, [128, 32 * 65]),
                ("kg", kg_d, [64, 64]),
                ("vg", vg_d, [128, 65]),
            ]:
                stg = const.tile(shape, f32, tag=name + "_s")
                rt = const.tile(shape, f32r, tag=name + "_r")
                half = shape[1] // 2
                if shape[1] >= 2048:
                    nc.sync.dma_start(out=stg[:, :half], in_=dh[:, :half])
                    nc.vector.tensor_copy(rt[:, :half], stg[:, :half])
                    nc.sync.dma_start(out=stg[:, half:], in_=dh[:, half:])
                    nc.vector.tensor_copy(rt[:, half:], stg[:, half:])
                else:
                    nc.sync.dma_start(out=stg[:], in_=dh[:])
                    nc.vector.tensor_copy(rt[:], stg[:])
                rtiles[name] = rt
            qt, kt, ve, kg, vg = (rtiles[k] for k in ["qt", "kt", "ve", "kg", "vg"])

            for s in range(NSB):
                out_ps = ps_o.tile([128, 512], f32, tag="out")
                qcol = 512 * s
                pv_jobs = []

                def band_tile(j0):
                    kti = (4 * (s - 1) + j0) if s >= 1 else (j0 - 4)
                    lower = j0 <= 3
                    sp0 = 0 if lower else 128 * (j0 - 4)
                    sps = ps_s.tile([128, 512], f32, tag="sps")
                    nc.tensor.matmul(
                        sps[:, sp0:512],
                        lhsT=kt[:, 128 * kti : 128 * kti + 128],
                        rhs=qt[:, qcol + sp0 : qcol + 512],
                        start=True,
                        stop=True,
                    )
                    if lower:
                        w = 512 - 128 * j0
                        nc.vector.tensor_add(
                            sps[:, 128 * j0 : 512],
                            sps[:, 128 * j0 : 512],
                            m_t[:, _M_OFF[j0] : _M_OFF[j0] + w],
                        )
                    E = ep.tile([128, 512], f32r, tag="E")
                    nc.scalar.activation(E[:, sp0:512], sps[:, sp0:512], AF.Exp, scale=SCALE)
                    if not lower:
                        # zero the non-causal triangle of the boundary block:
                        # keep where (rr - kk) >= 0
                        nc.gpsimd.affine_select(
                            out=E[:, sp0 : sp0 + 128],
                            in_=E[:, sp0 : sp0 + 128],
                            compare_op=mybir.AluOpType.is_ge,
                            fill=0.0,
                            base=0,
                            pattern=[[1, 128]],
                            channel_multiplier=-1,
                        )
                    pv_jobs.append((kti, sp0, E))

                band_tile(4)
                for j0 in ([0, 1, 2, 3] if s >= 1 else []) + [5, 6, 7]:
                    band_tile(j0)

                ng = 8 * (s - 1) if s >= 2 else 0
                Eg = None
                if ng > 0:
                    spg = ps_g.tile([128, 512], f32, tag="spg")
                    nc.tensor.matmul(
                        spg[0:ng, :],
                        lhsT=kg[:, 0:ng],
                        rhs=qt[:, qcol : qcol + 512],
                        start=True,
                        stop=True,
                    )
                    Eg = ep.tile([128, 512], f32r, tag="Eg")
                    nc.scalar.activation(Eg[0:ng, :], spg[0:ng, :], AF.Exp, scale=SCALE)

                n_pv = len(pv_jobs) + (1 if ng > 0 else 0)
                for i, (kti, sp0, E) in enumerate(pv_jobs):
                    nc.tensor.matmul(
                        out_ps[0:65, sp0:512],
                        lhsT=ve[:, 65 * kti : 65 * kti + 65],
                        rhs=E[:, sp0:512],
                        start=(i == 0),
                        stop=(i == n_pv - 1),
                    )
                if ng > 0:
                    nc.tensor.matmul(
                        out_ps[0:65, :],
                        lhsT=vg[0:ng, 0:65],
                        rhs=Eg[0:ng, :],
                        start=False,
                        stop=True,
                    )

                o_sb = op.tile([128, 512], f32, tag="o_sb")
                nc.vector.tensor_copy(o_sb[0:65, :], out_ps[0:65, :])
                nc.sync.dma_start(out=o_d[s], in_=o_sb[0:65, :])

    nc.compile()
    return nc


_CACHE = {}


def _get_nc():
    if "nc" not in _CACHE:
        _CACHE["nc"] = _build_nc()
    return _CACHE["nc"]


def kernel(Q, K, V):
    global LAST_RESULT
    Q = np.ascontiguousarray(np.asarray(Q), dtype=np.float32)
    K = np.ascontiguousarray(np.asarray(K), dtype=np.float32)
    V = np.ascontiguousarray(np.asarray(V), dtype=np.float32)
    B, H, t, d = Q.shape
    assert (B, H, t, d) == (1, 8, T, D)

    masks = _build_masks()

    nc = _get_nc()
    in_maps = []
    for h in range(8):
        q = Q[0, h]
        k = K[0, h]
        v = V[0, h]
        qt2 = np.ascontiguousarray(q.T)      # [64, 4096]
        kt2 = np.ascontiguousarray(k.T)
        ve = np.ones((128, 32 * 65), np.float32)
        vv = v.reshape(32, 128, 64).transpose(1, 0, 2)  # [128, 32, 64]
        ve3 = ve.reshape(128, 32, 65)
        ve3[:, :, :64] = vv
        kg = np.ascontiguousarray(k[::GS, :].T)  # [64, 64] = [d, g]
        vg = np.zeros((128, 65), np.float32)
        vg[:64, :64] = v[::GS, :]
        vg[:64, 64] = 1.0
        in_maps.append(
            dict(qt=qt2, kt=kt2, ve=np.ascontiguousarray(ve), kg=kg, vg=vg,
                 m=masks)
        )

    res = run_bass_kernel_spmd(nc, in_maps, list(range(8)), trace=TRACE)
    LAST_RESULT = res

    out = np.empty((1, 8, T, D), np.float32)
    for h in range(8):
        O = res.results[h]["o"]  # [NSB, 65, 512]
        for s in range(NSB):
            out[0, h, 512 * s : 512 * (s + 1), :] = (O[s, :64, :] / O[s, 64:65, :]).T
    return out
